# revision 1
# baseline (speedup 1.0000x reference)
"""Trainium2 Bass kernel for nn_ClassLoss (YOLO-style classification CE loss).

Strategy: the loss depends only on grid cells hit by valid target boxes
(<=50 cells/batch out of 4096). Each cell corresponds to 3 consecutive
"flat rows" of the [12288, 85] logits block (765 contiguous floats in DRAM).
So instead of streaming 127MB of logits, each core:
  1. loads its 4 batches' targets,
  2. computes per-box (row, col, class, valid), resolves last-write-wins
     duplicates with a pairwise comparison (block-diagonal across batches),
  3. indirect-DMA-gathers the needed cell blocks (two [100, 255] gathers,
     batches stacked in pairs along the partition axis),
  4. computes logsumexp over the 80 classes for the 3 rows of each cell and
     the label logit via a one-hot dot, masked by the winner flags,
  5. reduces to per-batch (loss_sum, cell_count) pairs via a selector matmul.
Host applies the per-batch mean (num / max(3*cnt,1)), sums across cores and
divides by the global batch size (the all-reduce + normalize of the
data-parallel sharding).
"""

import sys

sys.path.insert(0, "/opt/trn_rl_repo")

import numpy as np

import concourse.bass as bass
import concourse.tile as tile
from concourse import bacc, mybir
from concourse.bass_utils import run_bass_kernel_spmd

# Problem constants (hardcoded per harness contract).
B, A, H, W, NC_CLS, M = 32, 3, 64, 64, 80, 50
N_CORES = 8
B_CORE = B // N_CORES          # 4 batches per core
CELLS = H * W                  # 4096 cells per batch
ROWLEN = 3 * (5 + NC_CLS)      # 255 floats per cell (3 anchor rows x 85)
P2 = 2 * M                     # 100 partitions: 2 batches x 50 boxes
FP32 = mybir.dt.float32
I32 = mybir.dt.int32
Alu = mybir.AluOpType
Act = mybir.ActivationFunctionType


def _host_consts():
    # cidx[*, a*85 + k] = k-5 for k in [5,85), else -1 (never matches a class)
    cidx = np.full((P2, ROWLEN), -1.0, dtype=np.float32)
    for a in range(3):
        cidx[:, a * 85 + 5 : (a + 1) * 85] = np.arange(NC_CLS, dtype=np.float32)
    # ut2[p, q] = 1 iff same 50-block and q%50 > p%50 (strictly-later box)
    blk = np.arange(P2) // M
    mi = np.arange(P2) % M
    ut2 = ((blk[:, None] == blk[None, :]) & (mi[None, :] > mi[:, None])).astype(
        np.float32
    )
    ident = np.eye(P2, dtype=np.float32)
    # cell offset per partition, per pair: batch = 2*j + p//50
    boff = np.empty((P2, 2), dtype=np.float32)
    for j in range(2):
        boff[:M, j] = (2 * j) * CELLS
        boff[M:, j] = (2 * j + 1) * CELLS
    # block selector for per-batch partition sums
    bsel = np.zeros((P2, 2), dtype=np.float32)
    bsel[:M, 0] = 1.0
    bsel[M:, 1] = 1.0
    return {"cidx": cidx, "ut2": ut2, "ident": ident, "boff": boff, "bsel": bsel}


def _build_kernel_body(tc, x_ap, t_ap, out_ap, cidx_ap, ut_ap, ident_ap, boff_ap, bsel_ap):
    nc = tc.nc
    from contextlib import ExitStack

    ctx = ExitStack()
    with ctx:
        consts = ctx.enter_context(tc.tile_pool(name="consts", bufs=1))
        work = ctx.enter_context(tc.tile_pool(name="work", bufs=3))
        gpool = ctx.enter_context(tc.tile_pool(name="gather", bufs=2))
        psum = ctx.enter_context(tc.tile_pool(name="psum", bufs=2, space="PSUM"))
        psumr = ctx.enter_context(tc.tile_pool(name="psumr", bufs=1, space="PSUM"))
        fpool = ctx.enter_context(tc.tile_pool(name="final", bufs=1))

        # ---- constants / persistent tiles ----
        cidx_t = consts.tile([P2, ROWLEN], FP32)
        nc.sync.dma_start(cidx_t[:], cidx_ap[:])
        ut_t = consts.tile([P2, P2], FP32)
        nc.sync.dma_start(ut_t[:], ut_ap[:])
        ident_t = consts.tile([P2, P2], FP32)
        nc.sync.dma_start(ident_t[:], ident_ap[:])
        boff_t = consts.tile([P2, 2], FP32)
        nc.sync.dma_start(boff_t[:], boff_ap[:])
        bsel_t = consts.tile([P2, 2], FP32)
        nc.sync.dma_start(bsel_t[:], bsel_ap[:])

        stats = fpool.tile([P2, 4], FP32)  # (num, cnt) per pair-column

        # all targets: [100, 2, 5]; partition p = batch-in-pair p//50, box p%50
        tgt_t = consts.tile([P2, 2 * 5], FP32)
        nc.sync.dma_start(
            tgt_t[:].rearrange("p (j f) -> p j f", f=5),
            t_ap.rearrange("(j bb) m f -> (bb m) j f", j=2),
        )

        MAGIC = 8388608.0  # 2^23

        for j in range(2):
            Tb = tgt_t[:].rearrange("p (j f) -> p j f", f=5)[:, j, :]
            cls = Tb[:, 0:1]

            # valid[m] = sum(|t|) > 0
            val1 = work.tile([P2, 1], FP32, tag="val1")
            nc.vector.tensor_reduce(
                val1[:], Tb, axis=mybir.AxisListType.X, op=Alu.add,
                apply_absolute_value=True,
            )
            valid = work.tile([P2, 1], FP32, tag="valid")
            nc.vector.tensor_scalar(valid[:], val1[:], 0.0, None, op0=Alu.is_gt)

            # (c, r) = floor((x, y)*64) fused on [100, 2]: exact branchless
            # floor via ri = RNE(v) (magic add/sub), floor = ri - (ri > v)
            v2 = work.tile([P2, 2], FP32, tag="v2")
            nc.vector.tensor_scalar(v2[:], Tb[:, 1:3], 64.0, None, op0=Alu.mult)
            ri2 = work.tile([P2, 2], FP32, tag="ri2")
            nc.vector.tensor_scalar(
                ri2[:], Tb[:, 1:3], 64.0, MAGIC, op0=Alu.mult, op1=Alu.add
            )
            nc.vector.tensor_scalar(ri2[:], ri2[:], MAGIC, None, op0=Alu.subtract)
            corr2 = work.tile([P2, 2], FP32, tag="corr2")
            nc.vector.tensor_tensor(corr2[:], ri2[:], v2[:], op=Alu.is_gt)
            fl2 = work.tile([P2, 2], FP32, tag="fl2")
            nc.vector.tensor_tensor(fl2[:], ri2[:], corr2[:], op=Alu.subtract)
            cc, rr = fl2[:, 0:1], fl2[:, 1:2]

            # cell = r*64 + c + batch_offset
            cellf = work.tile([P2, 1], FP32, tag="cellf")
            nc.vector.scalar_tensor_tensor(
                cellf[:], rr, 64.0, cc, op0=Alu.mult, op1=Alu.add
            )
            celli = work.tile([P2, 1], I32, tag="celli")
            nc.vector.tensor_tensor(
                celli[:], cellf[:], boff_t[:, j : j + 1], op=Alu.add
            )

            # ---- gather the 100 cell blocks [100, 255] ASAP so the DMA and
            # exp overlap the winner resolution below ----
            graw = gpool.tile([P2, ROWLEN], FP32, tag="graw")
            nc.gpsimd.indirect_dma_start(
                out=graw[:],
                out_offset=None,
                in_=x_ap,
                in_offset=bass.IndirectOffsetOnAxis(ap=celli[:, :1], axis=0),
            )
            gv = graw[:].rearrange("p (a f) -> p a f", a=3)[:, :, 5:]
            ex = gpool.tile([P2, 3 * NC_CLS], FP32, tag="ex")
            nc.scalar.activation(
                ex[:].rearrange("p (a f) -> p a f", f=NC_CLS), gv, Act.Exp
            )

            # ---- winner resolution (last valid write wins) ----
            # key = valid ? cell : -1 so invalid boxes never match any cell
            key = work.tile([P2, 1], FP32, tag="key")
            nc.vector.scalar_tensor_tensor(
                key[:], cellf[:], 1.0, valid[:], op0=Alu.add, op1=Alu.mult
            )
            nc.vector.tensor_scalar(key[:], key[:], -1.0, None, op0=Alu.add)

            qT = psum.tile([P2, P2], FP32, tag="qT", space="PSUM")
            nc.tensor.transpose(qT[:], key[:].to_broadcast([P2, P2]), ident_t[:])

            same = work.tile([P2, P2], FP32, tag="same")
            nc.vector.tensor_scalar(same[:], qT[:], key[:], None, op0=Alu.is_equal)
            scrap0 = work.tile([P2, P2], FP32, tag="scrap0")
            coll = work.tile([P2, 1], FP32, tag="coll")
            nc.gpsimd.tensor_tensor(scrap0[:], same[:], ut_t[:], op=Alu.mult)
            nc.vector.tensor_reduce(
                coll[:], scrap0[:], axis=mybir.AxisListType.X, op=Alu.add
            )
            winner = work.tile([P2, 1], FP32, tag="winner")
            nc.vector.scalar_tensor_tensor(
                winner[:], coll[:], 0.0, valid[:], op0=Alu.is_equal, op1=Alu.mult
            )

            # ---- per-cell CE pieces ----
            se = work.tile([P2, 3], FP32, tag="se")
            nc.vector.tensor_reduce(
                se[:], ex[:].rearrange("p (a f) -> p a f", f=NC_CLS),
                axis=mybir.AxisListType.X, op=Alu.add,
            )
            lse = work.tile([P2, 3], FP32, tag="lse")
            nc.scalar.activation(lse[:], se[:], Act.Ln)
            s3 = work.tile([P2, 1], FP32, tag="s3")
            nc.vector.tensor_reduce(
                s3[:], lse[:], axis=mybir.AxisListType.X, op=Alu.add
            )

            # label logit sum over the 3 rows: one-hot dot against cidx
            ohc = work.tile([P2, ROWLEN], FP32, tag="ohc")
            nc.gpsimd.tensor_scalar(ohc[:], cidx_t[:], cls, None, op0=Alu.is_equal)
            scrap1 = work.tile([P2, ROWLEN], FP32, tag="scrap1")
            nc.gpsimd.tensor_tensor(scrap1[:], ohc[:], graw[:], op=Alu.mult)
            g3 = work.tile([P2, 1], FP32, tag="g3")
            nc.vector.tensor_reduce(
                g3[:], scrap1[:], axis=mybir.AxisListType.X, op=Alu.add
            )

            # d = (lse_sum - label_logit_sum); stats cols: num = winner*d, cnt = winner
            d = work.tile([P2, 1], FP32, tag="d")
            nc.vector.tensor_tensor(d[:], s3[:], g3[:], op=Alu.subtract)
            nc.vector.tensor_tensor(
                stats[:, 2 * j : 2 * j + 1], d[:], winner[:], op=Alu.mult
            )
            nc.vector.tensor_copy(stats[:, 2 * j + 1 : 2 * j + 2], winner[:])

        # ---- per-batch partition sums via PE: red[i, 2j+k] = batch 2j+i ----
        red = psumr.tile([2, 4], FP32, tag="red", space="PSUM")
        nc.tensor.matmul(red[:], bsel_t[:], stats[:], start=True, stop=True)
        fin = fpool.tile([2, 4], FP32)
        nc.vector.tensor_copy(fin[:], red[:])
        nc.sync.dma_start(out_ap[:], fin[:])


_CACHE = {}


def _get_compiled():
    if "nc" in _CACHE:
        return _CACHE["nc"]
    nc = bacc.Bacc(
        "TRN2",
        target_bir_lowering=False,
        debug=False,
        enable_asserts=False,
        num_devices=N_CORES,
    )
    x = nc.dram_tensor("xflat", [B_CORE * CELLS, ROWLEN], FP32, kind="ExternalInput")
    t = nc.dram_tensor("tgt", [B_CORE, M, 5], FP32, kind="ExternalInput")
    cidx = nc.dram_tensor("cidx", [P2, ROWLEN], FP32, kind="ExternalInput")
    ut2 = nc.dram_tensor("ut2", [P2, P2], FP32, kind="ExternalInput")
    ident = nc.dram_tensor("ident", [P2, P2], FP32, kind="ExternalInput")
    boff = nc.dram_tensor("boff", [P2, 2], FP32, kind="ExternalInput")
    bsel = nc.dram_tensor("bsel", [P2, 2], FP32, kind="ExternalInput")
    out = nc.dram_tensor("statsout", [2, 4], FP32, kind="ExternalOutput")

    with tile.TileContext(nc) as tc:
        _build_kernel_body(
            tc, x.ap(), t.ap(), out.ap(), cidx.ap(), ut2.ap(), ident.ap(),
            boff.ap(), bsel.ap(),
        )
    nc.compile()
    _CACHE["nc"] = nc
    return nc


def _finish(stats_list):
    """Host: per-batch mean, then mean over global batch (float64)."""
    total = 0.0
    for st in stats_list:
        st = np.asarray(st, dtype=np.float64)  # [2, 4]
        for j in range(2):
            for i in range(2):
                num = st[i, 2 * j]
                cnt = st[i, 2 * j + 1]
                total += num / max(3.0 * cnt, 1.0)
    return total / B


def _run(output, targets, trace=False):
    nc = _get_compiled()
    consts = _host_consts()
    output = np.ascontiguousarray(output, dtype=np.float32)
    targets = np.ascontiguousarray(targets, dtype=np.float32)
    in_maps = []
    for k in range(N_CORES):
        in_maps.append(
            {
                "xflat": output[k * B_CORE : (k + 1) * B_CORE].reshape(
                    B_CORE * CELLS, ROWLEN
                ),
                "tgt": targets[k * B_CORE : (k + 1) * B_CORE],
                **consts,
            }
        )
    res = run_bass_kernel_spmd(nc, in_maps, core_ids=list(range(N_CORES)), trace=trace)
    total = _finish([r["statsout"] for r in res.results])
    return np.float32(total), res


def kernel(output, targets):
    val, _ = _run(output, targets)
    return np.asarray(val, dtype=np.float32)



# revision 3
# speedup vs baseline: 1.5952x; 1.5952x over previous
"""Trainium2 Bass kernel for nn_ClassLoss (YOLO-style classification CE loss).

Strategy: the loss depends only on grid cells hit by valid target boxes
(<=50 cells/batch out of 4096). Each cell corresponds to 3 consecutive
"flat rows" of the [12288, 85] logits block (255 contiguous floats in DRAM).
Each core handles 4 batches packed as 100 partitions (2 blocks of 50 boxes)
x 2 pair-columns (j):
  1. one DMA brings all targets (+ per-partition batch offsets),
  2. cell indices for both pair-columns are computed in one [100,2] chain,
  3. ONE indirect DMA gathers all 200 cell blocks into [100, 2*255],
  4. winner resolution (last valid write wins) via globally-unique keys,
     a PE transpose, and a fused compare*mask+accumulate per pair-column,
  5. CE pieces: exp over the 480 class logits, sum-reduce, product over the
     3 anchors, one Ln, label logits via fused onehot-dot (scalar_tensor_
     tensor with accum), d = lse_sum - label_sum.
Host applies the per-batch mean (num / max(3*cnt,1)), sums across cores and
divides by the global batch size (the all-reduce + normalize of the
data-parallel sharding).
"""

import sys

sys.path.insert(0, "/opt/trn_rl_repo")

import numpy as np

import concourse.bass as bass
import concourse.tile as tile
from concourse import bacc, mybir
from concourse.bass_utils import run_bass_kernel_spmd

# Problem constants (hardcoded per harness contract).
B, A, H, W, NC_CLS, M = 32, 3, 64, 64, 80, 50
N_CORES = 8
B_CORE = B // N_CORES          # 4 batches per core
CELLS = H * W                  # 4096 cells per batch
ROWLEN = 3 * (5 + NC_CLS)      # 255 floats per cell (3 anchor rows x 85)
P2 = 2 * M                     # 100 partitions: 2 blocks x 50 boxes
NCON = ROWLEN + P2 + P2        # packed consts: cidx | UT | ident
FP32 = mybir.dt.float32
I32 = mybir.dt.int32
Alu = mybir.AluOpType
Act = mybir.ActivationFunctionType
MAGIC = 8388608.0  # 2^23


def _host_consts():
    # cidx[*, a*85 + 5 + k] = k for k in [0,80), else -1 (never matches a class)
    cidx = np.full((P2, ROWLEN), -1.0, dtype=np.float32)
    for a in range(3):
        cidx[:, a * 85 + 5 : (a + 1) * 85] = np.arange(NC_CLS, dtype=np.float32)
    # global strict upper triangle (keys are globally unique per j-column)
    q = np.arange(P2)
    ut = (q[None, :] > q[:, None]).astype(np.float32)
    ident = np.eye(P2, dtype=np.float32)
    return np.concatenate([cidx, ut, ident], axis=1)  # [100, NCON]


_CONSTS = _host_consts()


def _pack_targets(t4):
    """t4: [4, 50, 5] -> [100, 2, 6]: (cls,x,y,w,h,boff) per pair-column."""
    bb = np.arange(P2) // M
    m = np.arange(P2) % M
    pack = np.zeros((P2, 2, 6), dtype=np.float32)
    for j in range(2):
        pack[:, j, 0:5] = t4[2 * j + bb, m]
        pack[:, j, 5] = ((2 * j + bb) * CELLS).astype(np.float32)
    return pack


def _build_kernel_body(tc, x_ap, t_ap, c_ap, out_ap):
    nc = tc.nc
    from contextlib import ExitStack

    ctx = ExitStack()
    with ctx:
        consts = ctx.enter_context(tc.tile_pool(name="consts", bufs=1))
        work = ctx.enter_context(tc.tile_pool(name="work", bufs=2))
        gpool = ctx.enter_context(tc.tile_pool(name="gather", bufs=1))
        psum = ctx.enter_context(tc.tile_pool(name="psum", bufs=2, space="PSUM"))
        fpool = ctx.enter_context(tc.tile_pool(name="final", bufs=1))

        # ---- input DMAs: targets gate the critical path, so they go first
        # on the sync queue; the const pack rides the tensor engine's queue
        # in parallel. ----
        tgt_t = consts.tile([P2, 2, 6], FP32)
        nc.sync.dma_start(tgt_t[:], t_ap[:])
        cst_t = consts.tile([P2, NCON], FP32)
        nc.scalar.dma_start(cst_t[:], c_ap[:])
        cidx = cst_t[:, 0:ROWLEN]
        ut = cst_t[:, ROWLEN : ROWLEN + P2]
        ident = cst_t[:, ROWLEN + P2 : NCON]

        stats = fpool.tile([P2, 4], FP32)  # cols: num j0, num j1, win j0, win j1

        # ---- cell indices for both pair-columns: floor(xy*64) via the
        # round-to-nearest magic trick (exact: xy*64 is exact, floor = rne
        # minus correction). ----
        xy4 = tgt_t[:, :, 1:3]  # [100, 2, 2] strided (x, y)
        v4 = work.tile([P2, 2, 2], FP32, tag="v4")
        nc.vector.tensor_scalar(v4[:], xy4, 64.0, None, op0=Alu.mult)
        ri4 = work.tile([P2, 2, 2], FP32, tag="ri4")
        nc.vector.tensor_scalar(ri4[:], xy4, 64.0, MAGIC, op0=Alu.mult, op1=Alu.add)
        nc.vector.tensor_scalar(ri4[:], ri4[:], MAGIC, None, op0=Alu.subtract)
        corr4 = work.tile([P2, 2, 2], FP32, tag="corr4")
        nc.vector.tensor_tensor(corr4[:], ri4[:], v4[:], op=Alu.is_gt)
        fl4 = work.tile([P2, 2, 2], FP32, tag="fl4")
        nc.vector.tensor_tensor(fl4[:], ri4[:], corr4[:], op=Alu.subtract)
        rows = fl4[:, :, 1:2].rearrange("p a b -> p (a b)")
        cols = fl4[:, :, 0:1].rearrange("p a b -> p (a b)")

        cellf2 = work.tile([P2, 2], FP32, tag="cellf2")
        nc.vector.scalar_tensor_tensor(
            cellf2[:], rows, 64.0, cols, op0=Alu.mult, op1=Alu.add
        )
        keyf2 = work.tile([P2, 2], FP32, tag="keyf2")
        nc.vector.tensor_tensor(
            keyf2[:], cellf2[:], tgt_t[:, :, 5:6].rearrange("p a b -> p (a b)"),
            op=Alu.add,
        )
        celli2 = work.tile([P2, 2], I32, tag="celli2")
        nc.vector.tensor_copy(celli2[:], keyf2[:])

        # ---- ONE indirect gather for all 200 cell blocks ----
        graw = gpool.tile([P2, 2, ROWLEN], FP32, tag="graw")
        nc.gpsimd.indirect_dma_start(
            out=graw[:].rearrange("p j f -> p (j f)"),
            out_offset=None,
            in_=x_ap,
            in_offset=bass.IndirectOffsetOnAxis(ap=celli2[:, 0:2], axis=0),
        )

        # ---- winner resolution (last valid write wins), off critical path.
        # key = (cell + batch*4096 + 1) * valid: globally unique per
        # j-column for valid boxes, 0 for invalid. ----
        val2 = work.tile([P2, 2], FP32, tag="val2")
        nc.vector.tensor_reduce(
            val2[:], tgt_t[:, :, 0:5], axis=mybir.AxisListType.X, op=Alu.add,
            apply_absolute_value=True,
        )
        valid2 = work.tile([P2, 2], FP32, tag="valid2")
        nc.vector.tensor_scalar(valid2[:], val2[:], 0.0, None, op0=Alu.is_gt)
        key2 = work.tile([P2, 2], FP32, tag="key2")
        nc.vector.scalar_tensor_tensor(
            key2[:], keyf2[:], 1.0, valid2[:], op0=Alu.add, op1=Alu.mult
        )

        coll2 = work.tile([P2, 2], FP32, tag="coll2")
        for j in range(2):
            qT = psum.tile([P2, P2], FP32, tag=f"qT{j}", space="PSUM")
            nc.tensor.transpose(
                qT[:], key2[:, j : j + 1].to_broadcast([P2, P2]), ident
            )
            scrapq = work.tile([P2, P2], FP32, tag=f"scrapq{j}")
            nc.vector.scalar_tensor_tensor(
                scrapq[:], qT[:], key2[:, j : j + 1], ut,
                op0=Alu.is_equal, op1=Alu.mult, accum_out=coll2[:, j : j + 1],
            )
        # winner = (no later valid box on same cell) & valid
        nc.vector.scalar_tensor_tensor(
            stats[:, 2:4], coll2[:], 0.0, valid2[:], op0=Alu.is_equal, op1=Alu.mult
        )

        # ---- label logit sums: fused onehot-dot with accumulate ----
        g32 = work.tile([P2, 2], FP32, tag="g32")
        scrapg = work.tile([P2, 2, ROWLEN], FP32, tag="scrapg")
        for j in range(2):
            nc.vector.scalar_tensor_tensor(
                scrapg[:, j, :], cidx, tgt_t[:, j, 0:1], graw[:, j, :],
                op0=Alu.is_equal, op1=Alu.mult, accum_out=g32[:, j : j + 1],
            )

        # ---- logsumexp pieces: exp all 480 class logits, per-(j,anchor)
        # sums, product over anchors, one Ln ----
        gv = graw[:].rearrange("p j (a f) -> p (j a) f", f=85)[:, :, 5:]
        ex = gpool.tile([P2, 6, NC_CLS], FP32, tag="ex")
        nc.scalar.activation(ex[:], gv, Act.Exp)
        se6 = work.tile([P2, 6], FP32, tag="se6")
        nc.vector.tensor_reduce(se6[:], ex[:], axis=mybir.AxisListType.X, op=Alu.add)
        pr2 = work.tile([P2, 2], FP32, tag="pr2")
        nc.vector.tensor_reduce(
            pr2[:], se6[:].rearrange("p (j a) -> p j a", a=3),
            axis=mybir.AxisListType.X, op=Alu.mult,
        )
        ln2 = work.tile([P2, 2], FP32, tag="ln2")
        nc.scalar.activation(ln2[:], pr2[:], Act.Ln)

        # d = sum_a lse - sum_a label_logit; num = winner * d
        d2 = work.tile([P2, 2], FP32, tag="d2")
        nc.vector.tensor_tensor(d2[:], ln2[:], g32[:], op=Alu.subtract)
        nc.vector.tensor_tensor(stats[:, 0:2], d2[:], stats[:, 2:4], op=Alu.mult)

        nc.sync.dma_start(out_ap[:], stats[:])


_CACHE = {}


def _get_compiled():
    if "nc" in _CACHE:
        return _CACHE["nc"]
    nc = bacc.Bacc(
        "TRN2",
        target_bir_lowering=False,
        debug=False,
        enable_asserts=False,
        num_devices=N_CORES,
    )
    x = nc.dram_tensor("xflat", [B_CORE * CELLS, ROWLEN], FP32, kind="ExternalInput")
    t = nc.dram_tensor("tgt", [P2, 2, 6], FP32, kind="ExternalInput")
    c = nc.dram_tensor("cpack", [P2, NCON], FP32, kind="ExternalInput")
    out = nc.dram_tensor("statsout", [P2, 4], FP32, kind="ExternalOutput")

    with tile.TileContext(nc) as tc:
        _build_kernel_body(tc, x.ap(), t.ap(), c.ap(), out.ap())
    nc.compile()
    _CACHE["nc"] = nc
    return nc


def _finish(stats_list):
    """Host: per-batch mean, then mean over global batch (float64)."""
    total = 0.0
    for st in stats_list:
        st = np.asarray(st, dtype=np.float64)  # [100, 4]
        for j in range(2):
            for bb in range(2):
                rows = slice(bb * M, (bb + 1) * M)
                num = st[rows, j].sum()
                cnt = st[rows, 2 + j].sum()
                total += num / max(3.0 * cnt, 1.0)
    return total / B


def _run(output, targets, trace=False):
    nc = _get_compiled()
    output = np.ascontiguousarray(output, dtype=np.float32)
    targets = np.ascontiguousarray(targets, dtype=np.float32)
    in_maps = []
    for k in range(N_CORES):
        in_maps.append(
            {
                "xflat": output[k * B_CORE : (k + 1) * B_CORE].reshape(
                    B_CORE * CELLS, ROWLEN
                ),
                "tgt": _pack_targets(targets[k * B_CORE : (k + 1) * B_CORE]),
                "cpack": _CONSTS,
            }
        )
    res = run_bass_kernel_spmd(nc, in_maps, core_ids=list(range(N_CORES)), trace=trace)
    total = _finish([r["statsout"] for r in res.results])
    return np.float32(total), res


def kernel(output, targets):
    val, _ = _run(output, targets)
    return np.asarray(val, dtype=np.float32)
